# revision 16
# baseline (speedup 1.0000x reference)
"""Locally-connected graph-conv kernel for Trainium2 (Bass/Tile), bf16.

Computes out[b,t,m] = sum_n x[b,t,n] * (S*W)[n,m] + bias[m] for
x [64, 2048, 208], W/S [208, 208], bias [208].

The ring-graph support S is a +-4 band (mod 208), so each half of the
output nodes only needs a 112-row slice of the contraction dim:
  block 0 (m 0..103):   n in {204..207} ++ {0..107}
  block 1 (m 104..207): n in {100..207} ++ {0..3}
(112 = 7x16 keeps the 16-engine DMA stripe perfectly balanced; a
non-multiple-of-16 partition count strands engines and loses ~20%.)

Tolerance is 2e-2 and bf16 end-to-end measures 4.5e-3 max rel err, so
the host pre-casts x and the masked weight to bf16 (halves HBM load
traffic), the kernel stores bf16 (halves store traffic), and the host
upcasts on gather. The 16 DMA engines saturate at ~18GB/s reads /
~22GB/s writes each with all 8 cores streaming (~300GB/s/core
aggregate), so the ~14MB/core of traffic bounds the kernel; to keep
the engines packed:
  - setup (wh/bias, 47KB) rides the Scalar ring first;
  - block-0 x loads + block-0 stores queue on the Sync ring, block-1
    loads + block-1 stores on the Scalar ring, stores strictly BEHIND
    the prefetched loads (FIFO = loads get engine priority, then the
    writes burst);
  - deep pools: the whole shard's x tiles and o tiles stay resident;
  - PSUM evictions alternate VectorE/ScalarE in 4-bank [104, 2048]
    groups (fewer instructions -> fewer semaphores -> shorter teardown
    semaphore-clear chain at kernel exit).

Data-parallel over 8 NeuronCores: each core gets 16384 rows of the
flattened x, host-pre-assembled into a bf16 [224, 16384] tensor (two
112-row halo blocks). Stores are unpadded [104, T]; the host
transposes/upcasts at gather.
"""

import numpy as np
from contextlib import ExitStack

import concourse.bacc as bacc
import concourse.mybir as mybir
import concourse.tile as tile
from concourse.bass_utils import run_bass_kernel_spmd

N = 208                      # nodes
HALF = 104                   # output nodes per block
K = 4                        # band half-width of S
NH = 2 * K + HALF            # 112 contraction rows per block (halo incl.)
N_CORES = 8
B, T = 64, 2048
ROWS_TOTAL = B * T           # 131072
SHARD = ROWS_TOTAL // N_CORES    # 16384 rows per core
TB = 512                     # moving-block columns per matmul (fp32 PSUM bank)
TE = 2048                    # max eviction group columns (4 PSUM banks)
# tapered head: small first chunk so the first store is ready early and
# the HBM write direction mixes with the reads almost immediately
CHUNKS = [1024, 3072, 4096, 4096, 4096]
assert sum(CHUNKS) == SHARD
N_CHUNKS = len(CHUNKS)

FP32 = mybir.dt.float32
BF16 = mybir.dt.bfloat16
BF16_NP = mybir.dt.np(BF16)

# halo row order (indices into the [208] node dim) for each block
ROWS0 = list(range(N - K, N)) + list(range(0, HALF + K))          # 112
ROWS1 = list(range(HALF - K, N)) + list(range(0, K))              # 112

_CACHE = {}
LAST_RESULTS = None          # BassKernelResults of the most recent run


def _kernel_body(tc):
    nc = tc.nc
    # rows 0:112 block0 halo, 112:224 block1 halo
    x_d = nc.dram_tensor("xh", [2 * NH, SHARD], BF16, kind="ExternalInput").ap()
    w_d = nc.dram_tensor("wh", [NH, N], BF16, kind="ExternalInput").ap()
    b_d = nc.dram_tensor("bias", [1, N], FP32, kind="ExternalInput").ap()
    o_d = nc.dram_tensor("outt", [N, SHARD], BF16, kind="ExternalOutput").ap()

    with ExitStack() as ctx:
        const = ctx.enter_context(tc.tile_pool(name="const", bufs=1))

        # Setup rides the Scalar HWDGE ring (sync ring's first item is x).
        wh = const.tile([NH, N], BF16, tag="wh")
        nc.scalar.dma_start(wh, w_d)
        bA = const.tile([HALF, 1], FP32, tag="bA")
        bB = const.tile([HALF, 1], FP32, tag="bB")
        b_col = b_d.rearrange("o n -> n o")
        nc.scalar.dma_start(bA, b_col[0:HALF, :])
        nc.scalar.dma_start(bB, b_col[HALF:N, :])
        wh0 = wh[:, 0:HALF]
        wh1 = wh[:, HALF:N]

        # full-shard prefetch AND full output residency: stores queue
        # BEHIND the loads on the same rings (FIFO load priority)
        x0p = ctx.enter_context(tc.tile_pool(name="x0p", bufs=N_CHUNKS))
        x1p = ctx.enter_context(tc.tile_pool(name="x1p", bufs=N_CHUNKS))
        o0p = ctx.enter_context(tc.tile_pool(name="o0p", bufs=N_CHUNKS))
        o1p = ctx.enter_context(tc.tile_pool(name="o1p", bufs=N_CHUNKS))
        psp = ctx.enter_context(tc.tile_pool(name="psp", bufs=2, space="PSUM"))

        # all loads on the Sync ring in chunk-completion order; the Scalar
        # ring carries ONLY setup + stores, so each store transfers the
        # moment its chunk is evicted -> HBM reads and writes mix early
        # (mixed direction sustains ~350GB/s vs ~250 reads-only)
        offs = [sum(CHUNKS[:c]) for c in range(N_CHUNKS)]
        xh0s, xh1s = [], []
        for c, (lo, tw) in enumerate(zip(offs, CHUNKS)):
            tsl = slice(lo, lo + tw)
            xh0 = x0p.tile([NH, tw], BF16, tag="xh0")
            xh1 = x1p.tile([NH, tw], BF16, tag="xh1")
            nc.sync.dma_start(xh0, x_d[0:NH, tsl])
            nc.sync.dma_start(xh1, x_d[NH : 2 * NH, tsl])
            xh0s.append(xh0)
            xh1s.append(xh1)

        ectr = 0
        for c, (lo, tw) in enumerate(zip(offs, CHUNKS)):
            tsl = slice(lo, lo + tw)
            o0_t = o0p.tile([HALF, tw], BF16, tag="o0")
            o1_t = o1p.tile([HALF, tw], BF16, tag="o1")
            for blk, (whe, xt, o_t, be) in enumerate(
                [(wh0, xh0s[c], o0_t, bA), (wh1, xh1s[c], o1_t, bB)]
            ):
                for g0 in range(0, tw, TE):
                    gw = min(TE, tw - g0)
                    ps = psp.tile([HALF, gw], FP32, tag="ps")
                    for m0 in range(0, gw, TB):
                        nc.tensor.matmul(
                            ps[:, m0 : m0 + TB],
                            whe,
                            xt[:, g0 + m0 : g0 + m0 + TB],
                            start=True,
                            stop=True,
                        )
                    osl = o_t[:, g0 : g0 + gw]
                    # alternate evictions between VectorE and ScalarE
                    if ectr % 2 == 0:
                        nc.vector.tensor_scalar_add(osl, ps, be)
                    else:
                        nc.scalar.add(osl, ps, be)
                    ectr += 1
            if c == N_CHUNKS - 1:
                # tail: last stores ride both rings in parallel
                nc.scalar.dma_start(o_d[0:HALF, tsl], o0_t)
                nc.sync.dma_start(o_d[HALF:N, tsl], o1_t)
            else:
                nc.scalar.dma_start(o_d[0:HALF, tsl], o0_t)
                nc.scalar.dma_start(o_d[HALF:N, tsl], o1_t)


def _build():
    nc = bacc.Bacc(
        "TRN2",
        target_bir_lowering=False,
        debug=False,
        num_devices=N_CORES,
    )
    with tile.TileContext(nc) as tc:
        _kernel_body(tc)
    nc.compile()
    return nc


def kernel(x, W, b, S):
    global LAST_RESULTS
    nc = _CACHE.get("nc")
    if nc is None:
        nc = _build()
        _CACHE["nc"] = nc

    xf = np.asarray(x, np.float32).reshape(ROWS_TOTAL, N)
    WS = np.asarray(S, np.float32) * np.asarray(W, np.float32)
    wh = np.empty((NH, N), BF16_NP)
    wh[:, 0:HALF] = WS[ROWS0][:, 0:HALF].astype(BF16_NP)
    wh[:, HALF:N] = WS[ROWS1][:, HALF:N].astype(BF16_NP)
    bf = np.ascontiguousarray(np.asarray(b, np.float32).reshape(1, N))

    xt = np.ascontiguousarray(xf.T).astype(BF16_NP)      # [208, 131072] bf16
    in_maps = []
    for i in range(N_CORES):
        sl = slice(i * SHARD, (i + 1) * SHARD)
        xh = np.empty((2 * NH, SHARD), BF16_NP)
        xh[0:NH] = xt[ROWS0, sl]
        xh[NH : 2 * NH] = xt[ROWS1, sl]
        in_maps.append({"xh": xh, "wh": wh, "bias": bf})
    res = run_bass_kernel_spmd(nc, in_maps, core_ids=list(range(N_CORES)))
    LAST_RESULTS = res
    out = np.empty((ROWS_TOTAL, N), np.float32)
    for i, r in enumerate(res.results):
        yt = r["outt"]                                   # [208, SHARD] bf16
        out[i * SHARD : (i + 1) * SHARD, :] = yt.T.astype(np.float32)
    return out.reshape(B, T, N)


# revision 18
# speedup vs baseline: 1.0704x; 1.0704x over previous
"""Locally-connected graph-conv kernel for Trainium2 (Bass/Tile), bf16.

Computes out[b,t,m] = sum_n x[b,t,n] * (S*W)[n,m] + bias[m] for
x [64, 2048, 208], W/S [208, 208], bias [208].

The ring-graph support S is a +-4 band (mod 208), so each half of the
output nodes only needs a 112-row slice of the contraction dim:
  block 0 (m 0..103):   n in {204..207} ++ {0..107}
  block 1 (m 104..207): n in {100..207} ++ {0..3}
(112 = 7x16 keeps the 16-engine DMA stripe perfectly balanced; a
non-multiple-of-16 partition count strands engines and loses ~20%.)

Tolerance is 2e-2 and bf16 end-to-end measures 4.5e-3 max rel err, so
the host pre-casts x and the masked weight to bf16 (halves HBM load
traffic), the kernel stores bf16 (halves store traffic), and the host
upcasts on gather. The 16 DMA engines saturate at ~18GB/s reads /
~22GB/s writes each with all 8 cores streaming (~300GB/s/core
aggregate), so the ~14MB/core of traffic bounds the kernel; to keep
the engines packed:
  - setup (wh/bias, 47KB) rides the Scalar ring first;
  - block-0 x loads + block-0 stores queue on the Sync ring, block-1
    loads + block-1 stores on the Scalar ring, stores strictly BEHIND
    the prefetched loads (FIFO = loads get engine priority, then the
    writes burst);
  - deep pools: the whole shard's x tiles and o tiles stay resident;
  - PSUM evictions alternate VectorE/ScalarE in 4-bank [104, 2048]
    groups (fewer instructions -> fewer semaphores -> shorter teardown
    semaphore-clear chain at kernel exit).

Data-parallel over 8 NeuronCores: each core gets 16384 rows of the
flattened x, host-pre-assembled into a bf16 [224, 16384] tensor (two
112-row halo blocks). Stores are unpadded [104, T]; the host
transposes/upcasts at gather.
"""

import numpy as np
from contextlib import ExitStack

import concourse.bacc as bacc
import concourse.mybir as mybir
import concourse.tile as tile
from concourse.bass_utils import run_bass_kernel_spmd

N = 208                      # nodes
HALF = 104                   # output nodes per block
K = 4                        # band half-width of S
NH = 2 * K + HALF            # 112 contraction rows per block (halo incl.)
N_CORES = 8
B, T = 64, 2048
ROWS_TOTAL = B * T           # 131072
SHARD = ROWS_TOTAL // N_CORES    # 16384 rows per core
TB = 512                     # moving-block columns per matmul (fp32 PSUM bank)
TE = 2048                    # eviction group columns (4 PSUM banks)
TOUT = 4096                  # t-columns per DMA chunk (0.92 MB bf16 loads)
N_CHUNKS = SHARD // TOUT     # 4
SUB = TOUT // TE             # 2 psum groups per chunk per block

FP32 = mybir.dt.float32
BF16 = mybir.dt.bfloat16
BF16_NP = mybir.dt.np(BF16)

# halo row order (indices into the [208] node dim) for each block
ROWS0 = list(range(N - K, N)) + list(range(0, HALF + K))          # 112
ROWS1 = list(range(HALF - K, N)) + list(range(0, K))              # 112

_CACHE = {}
LAST_RESULTS = None          # BassKernelResults of the most recent run


def _kernel_body(tc):
    nc = tc.nc
    # rows 0:112 block0 halo, 112:224 block1 halo
    x_d = nc.dram_tensor("xh", [2 * NH, SHARD], BF16, kind="ExternalInput").ap()
    w_d = nc.dram_tensor("wh", [NH, N], BF16, kind="ExternalInput").ap()
    b_d = nc.dram_tensor("bias", [1, N], FP32, kind="ExternalInput").ap()
    o_d = nc.dram_tensor("outt", [N, SHARD], BF16, kind="ExternalOutput").ap()

    with ExitStack() as ctx:
        const = ctx.enter_context(tc.tile_pool(name="const", bufs=1))

        # Setup rides the Scalar HWDGE ring (sync ring's first item is x).
        wh = const.tile([NH, N], BF16, tag="wh")
        nc.scalar.dma_start(wh, w_d)
        bA = const.tile([HALF, 1], FP32, tag="bA")
        bB = const.tile([HALF, 1], FP32, tag="bB")
        b_col = b_d.rearrange("o n -> n o")
        nc.scalar.dma_start(bA, b_col[0:HALF, :])
        nc.scalar.dma_start(bB, b_col[HALF:N, :])
        wh0 = wh[:, 0:HALF]
        wh1 = wh[:, HALF:N]

        # full-shard prefetch AND full output residency: stores queue
        # BEHIND the loads on the same rings (FIFO load priority)
        x0p = ctx.enter_context(tc.tile_pool(name="x0p", bufs=N_CHUNKS))
        x1p = ctx.enter_context(tc.tile_pool(name="x1p", bufs=N_CHUNKS))
        o0p = ctx.enter_context(tc.tile_pool(name="o0p", bufs=N_CHUNKS))
        o1p = ctx.enter_context(tc.tile_pool(name="o1p", bufs=N_CHUNKS))
        psp = ctx.enter_context(tc.tile_pool(name="psp", bufs=2, space="PSUM"))

        xh0s, xh1s = [], []
        for c in range(N_CHUNKS):
            tsl = slice(c * TOUT, (c + 1) * TOUT)
            xh0 = x0p.tile([NH, TOUT], BF16, tag="xh0")
            if c == 0:
                # head: first matmul only needs the first columns
                h = TOUT // 2
                nc.sync.dma_start(xh0[:, 0:h], x_d[0:NH, 0:h])
                nc.sync.dma_start(xh0[:, h:TOUT], x_d[0:NH, h:TOUT])
            else:
                nc.sync.dma_start(xh0, x_d[0:NH, tsl])
            xh0s.append(xh0)
        for c in range(N_CHUNKS):
            tsl = slice(c * TOUT, (c + 1) * TOUT)
            xh1 = x1p.tile([NH, TOUT], BF16, tag="xh1")
            nc.scalar.dma_start(xh1, x_d[NH : 2 * NH, tsl])
            xh1s.append(xh1)

        for c in range(N_CHUNKS):
            tsl = slice(c * TOUT, (c + 1) * TOUT)
            o0_t = o0p.tile([HALF, TOUT], BF16, tag="o0")
            o1_t = o1p.tile([HALF, TOUT], BF16, tag="o1")
            for blk, (whe, xt, o_t, be) in enumerate(
                [(wh0, xh0s[c], o0_t, bA), (wh1, xh1s[c], o1_t, bB)]
            ):
                for s in range(SUB):
                    ps = psp.tile([HALF, TE], FP32, tag="ps")
                    for m0 in range(0, TE, TB):
                        g0 = s * TE + m0
                        nc.tensor.matmul(
                            ps[:, m0 : m0 + TB],
                            whe,
                            xt[:, g0 : g0 + TB],
                            start=True,
                            stop=True,
                        )
                    osl = o_t[:, s * TE : (s + 1) * TE]
                    # alternate evictions between VectorE and ScalarE
                    if (blk + s) % 2 == 0:
                        nc.vector.tensor_scalar_add(osl, ps, be)
                    else:
                        nc.scalar.add(osl, ps, be)
            # stores queue behind this ring's loads: strict load priority
            nc.sync.dma_start(o_d[0:HALF, tsl], o0_t)
            nc.scalar.dma_start(o_d[HALF:N, tsl], o1_t)


def _build():
    nc = bacc.Bacc(
        "TRN2",
        target_bir_lowering=False,
        debug=False,
        num_devices=N_CORES,
    )
    with tile.TileContext(nc) as tc:
        _kernel_body(tc)
    nc.compile()
    return nc


def kernel(x, W, b, S):
    global LAST_RESULTS
    nc = _CACHE.get("nc")
    if nc is None:
        nc = _build()
        _CACHE["nc"] = nc

    xf = np.asarray(x, np.float32).reshape(ROWS_TOTAL, N)
    WS = np.asarray(S, np.float32) * np.asarray(W, np.float32)
    wh = np.empty((NH, N), BF16_NP)
    wh[:, 0:HALF] = WS[ROWS0][:, 0:HALF].astype(BF16_NP)
    wh[:, HALF:N] = WS[ROWS1][:, HALF:N].astype(BF16_NP)
    bf = np.ascontiguousarray(np.asarray(b, np.float32).reshape(1, N))

    xt = np.ascontiguousarray(xf.T).astype(BF16_NP)      # [208, 131072] bf16
    in_maps = []
    for i in range(N_CORES):
        sl = slice(i * SHARD, (i + 1) * SHARD)
        xh = np.empty((2 * NH, SHARD), BF16_NP)
        xh[0:NH] = xt[ROWS0, sl]
        xh[NH : 2 * NH] = xt[ROWS1, sl]
        in_maps.append({"xh": xh, "wh": wh, "bias": bf})
    res = run_bass_kernel_spmd(nc, in_maps, core_ids=list(range(N_CORES)))
    LAST_RESULTS = res
    out = np.empty((ROWS_TOTAL, N), np.float32)
    for i, r in enumerate(res.results):
        yt = r["outt"]                                   # [208, SHARD] bf16
        out[i * SHARD : (i + 1) * SHARD, :] = yt.T.astype(np.float32)
    return out.reshape(B, T, N)
